# revision 1
# baseline (speedup 1.0000x reference)
"""Causal single-head attention (B=4, T=4096, C=1024, D=64) on 8 NeuronCores.

Sharding: core c = (batch b = c % 4, half h = c // 4).
Each core handles ALL queries of its batch against its half of the key
blocks (256-token blocks with block index ≡ h mod 2).  Pure SPMD: the
program is identical on every core; cores differ only in input data
(x[b]^T, block-pair-permuted for h=1, and the causal mask).  Each core
emits unnormalized U[q, 0:64] = sum_k exp(s) v and U[q, 64] = sum_k
exp(s); the host adds the two halves per batch and normalizes.

On-chip dataflow (bf16 compute, f32 PSUM accumulation):
  per 128-token tile tt:  PJ [128t, 192] = sum_c x_tile[c,tt]^T @ w[c]
    (fused Q|K|V projection, x tile stationary; non-key tiles: Q only)
  Q^T, K^T via PE transposes (deferred one unit for pipelining);
  V kept in [token, d] layout + ones column (softmax denominator)
  per query supertile st (512 q), local key tile pairs jp (diag first):
    S^T [128k, 2x512q] = K^T[j] @ Q^T[st]     (contraction over D=64)
    P^T = exp(S^T / 8)                        (one ACT instr per pair)
    diagonal pair: P^T *= mask                (DVE)
    U [128q, 65] += P^T-chunk^T @ [V_j | 1]   (P chunk stationary,
        deferred TWO pairs so the PE stays off the exp critical path)
  Projection units of the NEXT supertile interleave with attention
  pairs as fillers.  U -> SBUF -> DMA out per supertile (SWDGE/Pool
  path, keeping HWDGE free for input streaming).
"""
import sys
import numpy as np
import ml_dtypes
from collections import deque

if "/opt/trn_rl_repo" not in sys.path:
    sys.path.insert(0, "/opt/trn_rl_repo")

import concourse.bacc as bacc
import concourse.mybir as mybir
from concourse import tile
from concourse import bass_utils

bf16 = mybir.dt.bfloat16
f32 = mybir.dt.float32
BF = ml_dtypes.bfloat16

B, T, C, D = 4, 4096, 1024, 64
NC_ = C // 128      # 8 c-tiles
NTT = T // 128      # 32 token tiles
NST = 8             # query supertiles (512 q each)
STQ = 512

_CACHE = {}


def _build():
    nc = bacc.Bacc(None, target_bir_lowering=False, debug=False, num_devices=8)

    # xq tile-packed: xq[:, 1024*tt + 128*c : +128] = x^T[c-tile, token tile tt]
    xq = nc.dram_tensor("xq", [128, NTT * 1024], bf16, kind="ExternalInput")
    w = nc.dram_tensor("w", [128, NC_ * 192], bf16, kind="ExternalInput")
    # aux2 = diag mask [0:768] | identity [768:896]
    aux2 = nc.dram_tensor("aux2", [128, 896], bf16, kind="ExternalInput")
    out = nc.dram_tensor("out", [128, NST * 260], bf16, kind="ExternalOutput")

    with tile.TileContext(nc) as tc:
        with tc.tile_pool(name="sb", bufs=1) as sb, \
             tc.tile_pool(name="qk", bufs=3) as qkp, \
             tc.tile_pool(name="pp", bufs=7) as pp, \
             tc.tile_pool(name="uo", bufs=2) as uop, \
             tc.tile_pool(name="ps", bufs=2, space="PSUM") as ps:

            # ---- resident tiles ----
            xq_sb = sb.tile([128, NTT * 1024], bf16, tag="xq")
            w_sb = sb.tile([128, NC_ * 192], bf16, tag="w")
            aux_sb = sb.tile([128, 896], bf16, tag="aux")
            msk_sb = aux_sb[:, 0:768]
            idn_sb = aux_sb[:, 768:896]
            qT = sb.tile([64, T], bf16, tag="qT")       # Q^T strip
            kT = sb.tile([64, T // 2], bf16, tag="kT")  # K^T local tiles
            v_sb = sb.tile([128, 16 * 65], bf16, tag="v")  # [V_j | 1] tiles

            # DMA order = consumption order: w, then one DMA per token tile
            # (each delivers ALL c-tiles of that tile, so projection unit tt
            # unblocks as soon as ITS chunk lands).
            nc.sync.dma_start(w_sb[:], w[:])
            for tt in range(NTT):
                nc.sync.dma_start(xq_sb[:, 1024 * tt:1024 * (tt + 1)],
                                  xq[:, 1024 * tt:1024 * (tt + 1)])
                if tt == 2:  # identity gates the first transposes (~chunk3)
                    nc.sync.dma_start(aux_sb[:], aux2[:])
            # ones columns of the V tiles (denominator trick)
            for j in range(16):
                nc.gpsimd.memset(v_sb[:, 65 * j + 64:65 * j + 65], 1.0)

            # PE p-state warmup: the PE ramps to full clock only after ~3us
            # of continuous busy; run junk matmuls on a zeroed tile right at
            # t=0 so the ramp completes before the first real projection.
            warm = sb.tile([128, 640], bf16, tag="warm")
            nc.vector.memset(warm[:], 0.0)
            # trigger the ACT Exp table load (1.28us) at t=0, not at the
            # first real exp
            nc.scalar.activation(warm[:, 0:1], warm[:, 0:1],
                                 mybir.ActivationFunctionType.Exp, scale=1.0)
            for i in range(5):
                wps = ps.tile([128, 512], f32, tag="s", name=f"warm{i}")
                nc.tensor.matmul(wps[:], warm[:, 0:128], warm[:, 128:640],
                                 start=True, stop=True)

            # ---- projection units, software-pipelined ----
            # mm-phase: fused projection matmuls into PSUM + DVE copies out.
            # tp-phase (transposes + Q^T/K^T copies) is deferred one unit so
            # the PE never waits on the just-issued DVE copy.
            proj_prev = [None]

            def xsrc(tt, c):
                return xq_sb[:, 1024 * tt + 128 * c:1024 * tt + 128 * (c + 1)]

            def proj_tp(state):
                tt, qk, tp = state
                is_key = (tt & 1) == 0
                if is_key:
                    j = tt // 2
                    nc.tensor.transpose(tp[:, 0:128], qk[:, 0:64], idn_sb)
                    nc.tensor.transpose(tp[:, 128:256], qk[:, 64:128], idn_sb)
                    nc.vector.tensor_copy(qT[:, 128 * tt:128 * (tt + 1)],
                                          tp[:, 0:128])
                    nc.vector.tensor_copy(kT[:, 128 * j:128 * (j + 1)],
                                          tp[:, 128:256])
                else:
                    nc.tensor.transpose(tp[:, 0:128], qk[:], idn_sb)
                    nc.vector.tensor_copy(qT[:, 128 * tt:128 * (tt + 1)],
                                          tp[:, 0:128])

            def proj_unit(tt):
                """Key-ness fixed to tt%2==0 (128-interleaved); h=1 cores get
                adjacent-tile-permuted inputs so their key tiles land on
                even positions."""
                is_key = (tt & 1) == 0
                if not is_key and tt % 2 == 1 and tt <= 19:
                    # The tile that gates attention(st): compute Q^T directly
                    # (w stationary), skipping the qk-copy + transpose hops,
                    # with the qT copy on ACT to dodge the DVE queue.
                    pjq = ps.tile([64, 128], f32, tag="pj", name=f"pjq{tt}", bufs=3)
                    for c in range(NC_):
                        nc.tensor.matmul(pjq[:], w_sb[:, 192 * c:192 * c + 64],
                                         xsrc(tt, c),
                                         start=(c == 0), stop=(c == NC_ - 1))
                    nc.vector.tensor_copy(qT[:, 128 * tt:128 * (tt + 1)],
                                          pjq[:])
                    return
                wid = 192 if is_key else 64
                pj = ps.tile([128, wid], f32, tag="pj", name=f"pj{tt}", bufs=3)
                for c in range(NC_):
                    nc.tensor.matmul(pj[:], xsrc(tt, c),
                                     w_sb[:, 192 * c:192 * c + wid],
                                     start=(c == 0), stop=(c == NC_ - 1))
                qk = qkp.tile([128, 128 if is_key else 64], bf16, tag="qk",
                              name=f"qk{tt}")
                if is_key:
                    j = tt // 2
                    nc.vector.tensor_copy(qk[:], pj[:, 0:128])
                    nc.vector.tensor_copy(v_sb[:, 65 * j:65 * j + 64],
                                          pj[:, 128:192])
                else:
                    nc.vector.tensor_copy(qk[:], pj[:])
                tp = ps.tile([64, 256], bf16, tag="pj", name=f"tp{tt}", bufs=3)
                if proj_prev[0] is not None:
                    proj_tp(proj_prev[0])
                proj_prev[0] = (tt, qk, tp)

            def proj_flush():
                if proj_prev[0] is not None:
                    proj_tp(proj_prev[0])
                    proj_prev[0] = None

            # ---- attention: one continuous pair stream across supertiles ----
            # Global software pipeline: the U matmuls of a pair are deferred
            # two pairs (possibly crossing into the next supertile) so the
            # PE never sits on the ACT exp critical path, and the ACT stream
            # has no supertile-boundary bubble.
            u_state = {}   # st -> (u4 tile, n_emitted)
            pendings = []  # (st, jp, p2)

            def emit_u_d(st, jp, p2, d):
                if st not in u_state:
                    u_state[st] = [ps.tile([128, 260], f32, tag="u",
                                           name=f"u{st}", bufs=1), 0]
                ent = u_state[st]
                u4 = ent[0]
                j = 2 * jp + d
                total = 8 * (st + 1) - 2  # diag d1 contributes only g=2,3
                if jp == st and d == 1:
                    gs = [(2, 512), (3, 640)]
                else:
                    gs = [(g, 512 * d + 128 * g) for g in range(4)]
                for g, lo in gs:
                    # start=True zeroes the WHOLE PSUM bank: set it only
                    # on the chronologically first matmul into u4.
                    nc.tensor.matmul(
                        u4[:, 65 * g:65 * (g + 1)],
                        p2[:, lo:lo + 128],
                        v_sb[:, 65 * j:65 * (j + 1)],
                        start=(ent[1] == 0),
                        stop=(ent[1] == total - 1),
                        skip_group_check=True)
                    ent[1] += 1

            def ship(st):
                ent = u_state[st]
                if ent[1] == 8 * (st + 1) - 2:  # supertile complete -> ship
                    uo_t = uop.tile([128, 260], bf16, tag="uo", name=f"uo{st}")
                    if st <= 3:
                        nc.scalar.activation(uo_t[:], ent[0][:],
                                             mybir.ActivationFunctionType.Copy)
                    else:
                        nc.vector.tensor_copy(uo_t[:], ent[0][:])
                    eng = nc.sync if st == NST - 1 else nc.gpsimd
                    eng.dma_start(out[:, 260 * st:260 * (st + 1)], uo_t[:])
                    del u_state[st]

            def emit_u(st, jp, p2):
                if jp == st:  # diagonal pair -> causal mask, deferred here so
                    # the DVE is free for critical copies at the boundary
                    nc.vector.tensor_mul(p2[:, 0:768], p2[:, 0:768], msk_sb)
                emit_u_d(st, jp, p2, 0)
                emit_u_d(st, jp, p2, 1)
                ship(st)

            def emit_pair(st, jp):
                qsl = slice(STQ * st, STQ * (st + 1))
                s2 = ps.tile([128, 1024], f32, tag="s", name=f"s{st}_{jp}")
                p2 = pp.tile([128, 1024], bf16, tag="p", name=f"p{st}_{jp}")
                if jp == st:
                    # diagonal pair: with 128-interleaved keys, the second
                    # tile is visible only to queries [256:512) for BOTH
                    # halves -> 768 live columns instead of 1024
                    nc.tensor.matmul(s2[:, 0:512],
                                     kT[:, 128 * 2 * jp:128 * (2 * jp + 1)],
                                     qT[:, qsl], start=True, stop=True)
                    nc.tensor.matmul(s2[:, 512:768],
                                     kT[:, 128 * (2 * jp + 1):128 * (2 * jp + 2)],
                                     qT[:, STQ * st + 256:STQ * (st + 1)],
                                     start=True, stop=True)
                    nc.scalar.activation(p2[:, 0:768], s2[:, 0:768],
                                         mybir.ActivationFunctionType.Exp,
                                         scale=0.125)
                else:
                    for d in range(2):
                        j = 2 * jp + d
                        nc.tensor.matmul(s2[:, 512 * d:512 * (d + 1)],
                                         kT[:, 128 * j:128 * (j + 1)],
                                         qT[:, qsl], start=True, stop=True)
                    nc.scalar.activation(p2[:], s2[:],
                                         mybir.ActivationFunctionType.Exp,
                                         scale=0.125)
                pendings.append((st, jp, p2))
                if len(pendings) > 5:
                    emit_u(*pendings.pop(0))

            def filler_q(qtr, half):
                return deque(
                    (lambda t: (lambda: proj_unit(t)))(8 * qtr + 4 * half + i)
                    for i in range(4))

            # Fillers (projection units of supertile st+1) are emitted inside
            # supertile st's pair loop, starting at the pair index where their
            # DMA chunk has arrived (the stream is DMA-paced early on).
            for tl in range(4):
                proj_unit(tl)
            schedule = [filler_q(0, 1), filler_q(1, 0), filler_q(1, 1),
                        filler_q(2, 0), filler_q(2, 1), filler_q(3, 0),
                        filler_q(3, 1), deque()]
            for st in range(8):
                fillers = schedule[st]
                proj_flush()  # this supertile's Q^T/K^T must be complete
                for pi, jp in enumerate([st] + list(range(st))):  # diag first
                    emit_pair(st, jp)
                    if fillers:
                        fillers.popleft()()
                while fillers:
                    fillers.popleft()()
            for pd in pendings:
                emit_u(*pd)
            pendings.clear()

    nc.compile()
    return nc


def _get_nc():
    if "nc" not in _CACHE:
        _CACHE["nc"] = _build()
    return _CACHE["nc"]


def kernel(x, Wq, Wk, Wv, _trace=False):
    x = np.asarray(x)
    nc = _get_nc()

    # Token permutation per half: the program treats EVEN 128-token tiles
    # as key tiles.  For h=1 cores we swap each adjacent tile pair so THEIR
    # key tiles land on even positions.
    tok = np.arange(T)
    perm1 = 128 * ((tok // 128) ^ 1) + tok % 128  # swap adjacent 128-tiles

    xT = np.ascontiguousarray(x.transpose(0, 2, 1)).astype(BF)   # [B, C, T]
    xT1 = np.ascontiguousarray(xT[:, :, perm1])

    w_all = np.concatenate([Wq, Wk, Wv], axis=1).astype(np.float32)  # [C, 192]
    w_packed = np.ascontiguousarray(
        w_all.reshape(NC_, 128, 192).transpose(1, 0, 2).reshape(128, NC_ * 192)
    ).astype(BF)
    idn = np.eye(128, dtype=BF)

    # Masks for the diagonal pair: program key tile A holds global tile
    # 4st+h, tile B holds 4st+2+h; program query quarter g holds global
    # tile 4st+(g^h).  Causal test on global ids:
    #   A: 128h + k <= 128(g^h) + i   (cols 0:512, all four quarters)
    #   B: 128(2+h) + k <= 128(g^h) + i   (cols 512:768, quarters g=2,3)
    i = np.arange(128)[None, :]
    k = np.arange(128)[:, None]
    masks = {}
    for h in range(2):
        colsA = [(128 * h + k <= 128 * (g ^ h) + i) for g in range(4)]
        colsB = [(128 * (2 + h) + k <= 128 * (g ^ h) + i) for g in (2, 3)]
        masks[h] = np.concatenate(colsA + colsB, axis=1).astype(BF)  # [128,768]

    def pack_tiles(xTb):
        # [C, T] -> [128, tt*1024 + c*128 + t]
        return np.ascontiguousarray(
            xTb.reshape(NC_, 128, NTT, 128).transpose(1, 2, 0, 3)
            .reshape(128, NTT * 1024))

    in_maps = []
    for c in range(8):
        b, h = c % 4, c // 4
        xTb = xT[b] if h == 0 else xT1[b]
        in_maps.append({
            "xq": pack_tiles(xTb),
            "w": w_packed,
            "aux2": np.concatenate([masks[h], idn], axis=1),
        })

    res = bass_utils.run_bass_kernel_spmd(nc, in_maps, core_ids=list(range(8)),
                                          trace=_trace)
    _CACHE["last_results"] = res

    # Decode: U[c] [128, 8*260] -> [q_perm, 65]; un-permute h=1 tokens.
    O = np.empty((B, T, D), dtype=np.float32)
    for b in range(B):
        Uh = []
        for h in range(2):
            U = np.asarray(res.results[b + 4 * h]["out"],
                           dtype=np.float32)            # [128, 2080]
            U = U.reshape(128, NST, 4, 65).transpose(1, 2, 0, 3)
            U = U.reshape(T, 65)                          # permuted q order
            Uh.append(U[perm1] if h == 1 else U)          # global q order
        Ut = Uh[0] + Uh[1]
        O[b] = Ut[:, 0:64] / Ut[:, 64:65]
    return O



# revision 2
# speedup vs baseline: 1.0035x; 1.0035x over previous
"""Causal single-head attention (B=4, T=4096, C=1024, D=64) on 8 NeuronCores.

Sharding: core c = (batch b = c % 4, half h = c // 4).
Each core handles ALL queries of its batch against its half of the key
blocks (256-token blocks with block index ≡ h mod 2).  Pure SPMD: the
program is identical on every core; cores differ only in input data
(x[b]^T, block-pair-permuted for h=1, and the causal mask).  Each core
emits unnormalized U[q, 0:64] = sum_k exp(s) v and U[q, 64] = sum_k
exp(s); the host adds the two halves per batch and normalizes.

On-chip dataflow (bf16 compute, f32 PSUM accumulation):
  Key tiles (even tt): ONE stacked matmul per c-tile with w stationary
    produces [Q^T; K^T] [128, 128t] directly (rows 0:64 = Q^T, 64:128 =
    K^T) -- no PE transposes; plus a fused V matmul [128t, 64] with x
    stationary.  DVE copies qT / kT (partition-shifted) / V out of PSUM.
  Q-only tiles (odd tt): supertiles 0,1 use the direct path (w
    stationary, [64, 128t] out) for short critical-path latency; later
    supertiles use the cheaper fused path ([128t, 64] out) with the two
    odd tiles of a supertile batched through ONE [128,128] PE transpose.
  per query supertile st (512 q), local key tile pairs jp (diag first):
    S^T [128k, 2x512q] = K^T[j] @ Q^T[st]     (contraction over D=64)
    P^T = exp(S^T / 8)                        (one ACT instr per pair)
    diagonal pair: P^T *= mask                (DVE)
    U [128q, 65] += P^T-chunk^T @ [V_j | 1]   (P chunk stationary,
        deferred TWO pairs so the PE stays off the exp critical path;
        the deferral queue is drained early during st=7 to cut the tail)
  Projection units of the NEXT supertile interleave with attention
  pairs as fillers.  U -> SBUF -> DMA out per supertile (SWDGE/Pool
  path for st<7, keeping HWDGE free for input streaming).
"""
import sys
import numpy as np
import ml_dtypes

if "/opt/trn_rl_repo" not in sys.path:
    sys.path.insert(0, "/opt/trn_rl_repo")

import concourse.bacc as bacc
import concourse.mybir as mybir
from concourse import tile
from concourse import bass_utils

bf16 = mybir.dt.bfloat16
f32 = mybir.dt.float32
BF = ml_dtypes.bfloat16

B, T, C, D = 4, 4096, 1024, 64
NC_ = C // 128      # 8 c-tiles
NTT = T // 128      # 32 token tiles
NST = 8             # query supertiles (512 q each)
STQ = 512

_CACHE = {}


def _build():
    nc = bacc.Bacc(None, target_bir_lowering=False, debug=False, num_devices=8)

    # xq tile-packed: xq[:, 1024*tt + 128*c : +128] = x^T[c-tile, token tile tt]
    xq = nc.dram_tensor("xq", [128, NTT * 1024], bf16, kind="ExternalInput")
    w = nc.dram_tensor("w", [128, NC_ * 192], bf16, kind="ExternalInput")
    # aux2 = diag mask [0:768] | identity [768:896]
    aux2 = nc.dram_tensor("aux2", [128, 896], bf16, kind="ExternalInput")
    out = nc.dram_tensor("out", [128, NST * 260], bf16, kind="ExternalOutput")

    with tile.TileContext(nc) as tc:
        with tc.tile_pool(name="sb", bufs=1) as sb, \
             tc.tile_pool(name="qk", bufs=3) as qkp, \
             tc.tile_pool(name="pp", bufs=7) as pp, \
             tc.tile_pool(name="uo", bufs=2) as uop, \
             tc.tile_pool(name="ps", bufs=2, space="PSUM") as ps:

            # ---- resident tiles ----
            xq_sb = sb.tile([128, NTT * 1024], bf16, tag="xq")
            w_sb = sb.tile([128, NC_ * 192], bf16, tag="w")
            aux_sb = sb.tile([128, 896], bf16, tag="aux")
            msk_sb = aux_sb[:, 0:768]
            idn_sb = aux_sb[:, 768:896]
            qT = sb.tile([64, T], bf16, tag="qT")       # Q^T strip
            kT = sb.tile([64, T // 2], bf16, tag="kT")  # K^T local tiles
            v_sb = sb.tile([128, 16 * 65], bf16, tag="v")  # [V_j | 1] tiles

            # DMA order = consumption order: w, then one DMA per token tile
            # (each delivers ALL c-tiles of that tile).
            nc.sync.dma_start(w_sb[:], w[:])
            for tt in range(NTT):
                nc.sync.dma_start(xq_sb[:, 1024 * tt:1024 * (tt + 1)],
                                  xq[:, 1024 * tt:1024 * (tt + 1)])
                if tt == 2:  # identity/masks gate the first diag pair (~st0)
                    nc.sync.dma_start(aux_sb[:], aux2[:])
            # exp-table trigger input on its OWN tile so the warm matmuls
            # don't serialize behind the 1.28us ACT table load
            trig = sb.tile([128, 1], bf16, tag="trig")
            nc.gpsimd.memset(trig[:], 0.0)
            # ones columns of the V tiles (denominator trick)
            for j in range(16):
                nc.gpsimd.memset(v_sb[:, 65 * j + 64:65 * j + 65], 1.0)

            # PE p-state warmup: the PE ramps to full clock only after ~3us
            # of continuous busy; run junk matmuls on a zeroed tile right at
            # t=0 so the ramp completes before the first real projection.
            warm = sb.tile([128, 640], bf16, tag="warm")
            nc.vector.memset(warm[:], 0.0)
            # trigger the ACT Exp table load (1.28us) at t=0, not at the
            # first real exp
            nc.scalar.activation(trig[:], trig[:],
                                 mybir.ActivationFunctionType.Exp, scale=1.0)
            for i in range(4):
                wps = ps.tile([128, 512], f32, tag="s", name=f"warm{i}")
                nc.tensor.matmul(wps[:], warm[:, 0:128], warm[:, 128:640],
                                 start=True, stop=True)

            def xsrc(tt, c):
                return xq_sb[:, 1024 * tt + 128 * c:1024 * tt + 128 * (c + 1)]

            # ---- projection units ----
            def key_unit(tt):
                """Stacked [Q^T; K^T] direct + fused V for an even tile."""
                j = tt // 2
                pj = ps.tile([128, 192], f32, tag="pj", name=f"pjk{tt}",
                             bufs=3)
                for c in range(NC_):
                    # w stationary: out rows 0:64 = Q^T, 64:128 = K^T
                    nc.tensor.matmul(pj[:, 0:128],
                                     w_sb[:, 192 * c:192 * c + 128],
                                     xsrc(tt, c),
                                     start=(c == 0), stop=(c == NC_ - 1),
                                     skip_group_check=True)
                for c in range(NC_):
                    # x stationary: V in [token, d] layout
                    nc.tensor.matmul(pj[:, 128:192], xsrc(tt, c),
                                     w_sb[:, 192 * c + 128:192 * c + 192],
                                     start=False, stop=(c == NC_ - 1),
                                     skip_group_check=True)
                nc.vector.tensor_copy(kT[:, 128 * j:128 * (j + 1)],
                                      pj[64:128, 0:128])
                nc.vector.tensor_copy(qT[:, 128 * tt:128 * (tt + 1)],
                                      pj[0:64, 0:128])
                nc.vector.tensor_copy(v_sb[:, 65 * j:65 * j + 64],
                                      pj[:, 128:192])

            def q_direct(tt):
                """Direct Q^T (w stationary) -- shortest latency chain."""
                pjq = ps.tile([64, 128], f32, tag="pj", name=f"pjq{tt}",
                              bufs=3)
                for c in range(NC_):
                    nc.tensor.matmul(pjq[:], w_sb[:, 192 * c:192 * c + 64],
                                     xsrc(tt, c),
                                     start=(c == 0), stop=(c == NC_ - 1))
                nc.vector.tensor_copy(qT[:, 128 * tt:128 * (tt + 1)], pjq[:])

            def q_pair_A(st):
                """Fused Q projection of tile 4st+1 into a shared pj bank."""
                t1 = 4 * st + 1
                pj = ps.tile([128, 128], f32, tag="pj", name=f"pjp{st}",
                             bufs=3)
                for c in range(NC_):
                    nc.tensor.matmul(pj[:, 0:64], xsrc(t1, c),
                                     w_sb[:, 192 * c:192 * c + 64],
                                     start=(c == 0), stop=(c == NC_ - 1),
                                     skip_group_check=True)
                return pj

            def q_pair_B(st, pj):
                """Tile 4st+3 projection + ONE batched transpose + copies."""
                t1, t2 = 4 * st + 1, 4 * st + 3
                for c in range(NC_):
                    nc.tensor.matmul(pj[:, 64:128], xsrc(t2, c),
                                     w_sb[:, 192 * c:192 * c + 64],
                                     start=False, stop=(c == NC_ - 1),
                                     skip_group_check=True)
                qk2 = qkp.tile([128, 128], bf16, tag="qk", name=f"qk{st}")
                nc.vector.tensor_copy(qk2[:], pj[:])
                tp = ps.tile([128, 128], bf16, tag="pj", name=f"tp{st}",
                             bufs=3)
                nc.tensor.transpose(tp[:], qk2[:], idn_sb)
                nc.vector.tensor_copy(qT[:, 128 * t1:128 * (t1 + 1)],
                                      tp[0:64, :])
                nc.vector.tensor_copy(qT[:, 128 * t2:128 * (t2 + 1)],
                                      tp[64:128, :])

            # ---- attention: one continuous pair stream across supertiles ----
            # Global software pipeline: the U matmuls of a pair are deferred
            # two pairs (possibly crossing into the next supertile) so the
            # PE never sits on the ACT exp critical path, and the ACT stream
            # has no supertile-boundary bubble.
            u_state = {}   # st -> (u4 tile, n_emitted)
            pendings = []  # (st, jp, p2)

            def emit_u_d(st, jp, p2, d):
                if st not in u_state:
                    u_state[st] = [ps.tile([128, 260], f32, tag="u",
                                           name=f"u{st}", bufs=1), 0]
                ent = u_state[st]
                u4 = ent[0]
                j = 2 * jp + d
                total = 8 * (st + 1) - 2  # diag d1 contributes only g=2,3
                if jp == st and d == 1:
                    gs = [(2, 512), (3, 640)]
                else:
                    gs = [(g, 512 * d + 128 * g) for g in range(4)]
                for g, lo in gs:
                    # start=True zeroes the WHOLE PSUM bank: set it only
                    # on the chronologically first matmul into u4.
                    nc.tensor.matmul(
                        u4[:, 65 * g:65 * (g + 1)],
                        p2[:, lo:lo + 128],
                        v_sb[:, 65 * j:65 * (j + 1)],
                        start=(ent[1] == 0),
                        stop=(ent[1] == total - 1),
                        skip_group_check=True)
                    ent[1] += 1

            def ship(st):
                ent = u_state[st]
                if ent[1] == 8 * (st + 1) - 2:  # supertile complete -> ship
                    uo_t = uop.tile([128, 260], bf16, tag="uo", name=f"uo{st}")
                    if st <= 3 or st == NST - 1:
                        nc.scalar.activation(uo_t[:], ent[0][:],
                                             mybir.ActivationFunctionType.Copy)
                    else:
                        nc.vector.tensor_copy(uo_t[:], ent[0][:])
                    eng = nc.sync if st == NST - 1 else nc.gpsimd
                    eng.dma_start(out[:, 260 * st:260 * (st + 1)], uo_t[:])
                    del u_state[st]

            def emit_u(st, jp, p2):
                if jp == st:  # diagonal pair -> causal mask, deferred here so
                    # the DVE is free for critical copies at the boundary
                    nc.vector.tensor_mul(p2[:, 0:768], p2[:, 0:768], msk_sb)
                emit_u_d(st, jp, p2, 0)
                emit_u_d(st, jp, p2, 1)
                ship(st)

            def emit_pair(st, jp, cap=5):
                qsl = slice(STQ * st, STQ * (st + 1))
                s2 = ps.tile([128, 1024], f32, tag="s", name=f"s{st}_{jp}")
                p2 = pp.tile([128, 1024], bf16, tag="p", name=f"p{st}_{jp}")
                if jp == st:
                    # diagonal pair: with 128-interleaved keys, the second
                    # tile is visible only to queries [256:512) for BOTH
                    # halves -> 768 live columns instead of 1024
                    nc.tensor.matmul(s2[:, 0:512],
                                     kT[:, 128 * 2 * jp:128 * (2 * jp + 1)],
                                     qT[:, qsl], start=True, stop=True)
                    nc.tensor.matmul(s2[:, 512:768],
                                     kT[:, 128 * (2 * jp + 1):128 * (2 * jp + 2)],
                                     qT[:, STQ * st + 256:STQ * (st + 1)],
                                     start=True, stop=True)
                    nc.scalar.activation(p2[:, 0:768], s2[:, 0:768],
                                         mybir.ActivationFunctionType.Exp,
                                         scale=0.125)
                else:
                    for d in range(2):
                        j = 2 * jp + d
                        nc.tensor.matmul(s2[:, 512 * d:512 * (d + 1)],
                                         kT[:, 128 * j:128 * (j + 1)],
                                         qT[:, qsl], start=True, stop=True)
                    nc.scalar.activation(p2[:], s2[:],
                                         mybir.ActivationFunctionType.Exp,
                                         scale=0.125)
                pendings.append((st, jp, p2))
                while len(pendings) > cap:
                    emit_u(*pendings.pop(0))

            # ---- schedule ----
            # Fillers (projection units of supertile st+1) are emitted inside
            # supertile st's pair loop in DMA-arrival order; None = no filler
            # after that pair.
            key_unit(0)
            q_direct(1)
            key_unit(2)
            q_direct(3)

            qp_live = {}  # st -> pj tile from q_pair_A

            def fillers_for(st):
                """Filler closures to interleave into pairs of supertile st-1
                (emitted in tile-arrival order)."""
                if st >= NST:
                    return []
                if st == 1:
                    return [lambda: key_unit(4), lambda: q_direct(5),
                            lambda: key_unit(6), lambda: q_direct(7)]
                fl = [lambda: key_unit(4 * st),
                      lambda: qp_live.__setitem__(st, q_pair_A(st)),
                      lambda: key_unit(4 * st + 2),
                      lambda: q_pair_B(st, qp_live.pop(st))]
                return fl

            for st in range(NST):
                fillers = fillers_for(st + 1)
                npairs = st + 1
                for pi, jp in enumerate([st] + list(range(st))):  # diag first
                    # during the last supertile, drain the deferral queue
                    # early so the tail after the final exp is short
                    cap = 5 if st < NST - 1 else max(1, 5 - pi)
                    emit_pair(st, jp, cap=cap)
                    if fillers:
                        fillers.pop(0)()
                while fillers:
                    fillers.pop(0)()
            for pd in pendings:
                emit_u(*pd)
            pendings.clear()

    nc.compile()
    return nc


def _get_nc():
    if "nc" not in _CACHE:
        _CACHE["nc"] = _build()
    return _CACHE["nc"]


def kernel(x, Wq, Wk, Wv, _trace=False):
    x = np.asarray(x)
    nc = _get_nc()

    # Token permutation per half: the program treats EVEN 128-token tiles
    # as key tiles.  For h=1 cores we swap each adjacent tile pair so THEIR
    # key tiles land on even positions.
    tok = np.arange(T)
    perm1 = 128 * ((tok // 128) ^ 1) + tok % 128  # swap adjacent 128-tiles

    xT = np.ascontiguousarray(x.transpose(0, 2, 1)).astype(BF)   # [B, C, T]
    xT1 = np.ascontiguousarray(xT[:, :, perm1])

    w_all = np.concatenate([Wq, Wk, Wv], axis=1).astype(np.float32)  # [C, 192]
    w_packed = np.ascontiguousarray(
        w_all.reshape(NC_, 128, 192).transpose(1, 0, 2).reshape(128, NC_ * 192)
    ).astype(BF)
    idn = np.eye(128, dtype=BF)

    # Masks for the diagonal pair: program key tile A holds global tile
    # 4st+h, tile B holds 4st+2+h; program query quarter g holds global
    # tile 4st+(g^h).  Causal test on global ids:
    #   A: 128h + k <= 128(g^h) + i   (cols 0:512, all four quarters)
    #   B: 128(2+h) + k <= 128(g^h) + i   (cols 512:768, quarters g=2,3)
    i = np.arange(128)[None, :]
    k = np.arange(128)[:, None]
    masks = {}
    for h in range(2):
        colsA = [(128 * h + k <= 128 * (g ^ h) + i) for g in range(4)]
        colsB = [(128 * (2 + h) + k <= 128 * (g ^ h) + i) for g in (2, 3)]
        masks[h] = np.concatenate(colsA + colsB, axis=1).astype(BF)  # [128,768]

    def pack_tiles(xTb):
        # [C, T] -> [128, tt*1024 + c*128 + t]
        return np.ascontiguousarray(
            xTb.reshape(NC_, 128, NTT, 128).transpose(1, 2, 0, 3)
            .reshape(128, NTT * 1024))

    in_maps = []
    for c in range(8):
        b, h = c % 4, c // 4
        xTb = xT[b] if h == 0 else xT1[b]
        in_maps.append({
            "xq": pack_tiles(xTb),
            "w": w_packed,
            "aux2": np.concatenate([masks[h], idn], axis=1),
        })

    res = bass_utils.run_bass_kernel_spmd(nc, in_maps, core_ids=list(range(8)),
                                          trace=_trace)
    _CACHE["last_results"] = res

    # Decode: U[c] [128, 8*260] -> [q_perm, 65]; un-permute h=1 tokens.
    O = np.empty((B, T, D), dtype=np.float32)
    for b in range(B):
        Uh = []
        for h in range(2):
            U = np.asarray(res.results[b + 4 * h]["out"],
                           dtype=np.float32)            # [128, 2080]
            U = U.reshape(128, NST, 4, 65).transpose(1, 2, 0, 3)
            U = U.reshape(T, 65)                          # permuted q order
            Uh.append(U[perm1] if h == 1 else U)          # global q order
        Ut = Uh[0] + Uh[1]
        O[b] = Ut[:, 0:64] / Ut[:, 64:65]
    return O
